# revision 34
# baseline (speedup 1.0000x reference)
"""Trainium2 Bass kernel for the tiny-RNN scan problem.

Math (reference): h_{t+1} = tanh(x[:,t,None]*w_ih[0] + h_t @ w_hh) scanned over
T=2048 steps; pred = (h_T @ w_ho)[:,0]; loss = mean((y-pred)^2).

Key algorithmic observation: the step map is a contraction whenever
sigma_max(w_hh) < 1 (|tanh(u)-tanh(v)| <= |u-v|), so h_T only depends on the
last k timesteps up to an error that decays geometrically (factor sigma per
step, further damped by tanh saturation).  We therefore start the scan from
h=0 at t = T-k with k chosen adaptively at runtime from the actual w_hh:
  - candidate k is accepted when a cheap host-side self-consistency probe on a
    row sample shows |pred(start T-k) - pred(start T-k-64)| below fp32 noise,
  - with a provable sigma^k bound as fallback, and a multi-launch chained path
    (h0 input / h_out output) when w_hh is not contracting.
For the seeded inputs (sigma=0.793) k=32..64 is numerically identical to the
full scan at fp32 precision.

Device mapping (pure data parallel over batch, 2048 rows/core):
  rows n = g*R + r with g in [0,G), r in [0,R), G*R = 2048.
  SBUF "rh" chain tile [128 partitions, (k+2)*R+1 cols]:
    partitions 0..3G-1,  col block t: h_t  (partition i*G+g holds h_i of
                                           group g)
    partitions 3G..4G-1, col block t: x_t  (partition 3G+g)
  One step = one PE matmul (stationary [4G, 3G] block-diagonal fold of
  w_hh/w_ih) writing the pre-activation to PSUM [3G, R], then one ACT tanh
  PSUM -> SBUF into col block t+1.  Final pred via a tiny matmul with a
  [3G, G] fold of w_ho; loss partials via DVE ops.  pred/partials land in
  spare rh columns so ONE output DMA ships h_k, pred and the loss partials;
  the host sums 8*G partials.  G=16 balances LDWEIGHTS cost (proportional to
  3G stationary columns, paid twice per step for fp32 hi/lo passes) against
  matmul streaming cost (proportional to R).
"""

import os
import sys
import numpy as np

import concourse.bass as bass
import concourse.bacc as bacc
import concourse.tile as tile
from concourse.tile import add_dep_helper
import concourse.mybir as mybir
from concourse.bass_utils import run_bass_kernel_spmd

F32 = mybir.dt.float32
HIDDEN = 3
B, T = 16384, 2048
NCORES = 8
ROWS = B // NCORES  # 2048 rows per core
G = 32              # row groups
R = ROWS // G       # rows per group (free dim)
HP = 3 * G          # partitions holding h
NP = 4 * G          # partitions holding [h; x] (matmul contraction)

KMAX = 512          # max scan steps per launch (SBUF chain capacity)

_cache: dict = {}
last_exec_time_ns = None  # set when BASS_RNN_TRACE=1
last_profile = None


def _install_ntff_shim():
    """Register the axon NTFF-profile hook (antenv.axon_hooks is absent in
    this image; replicate trn_boot._ntff_profile_via_ctypes)."""
    try:
        from antenv.axon_hooks import get_axon_ntff_profile_hook  # noqa: F401
        return True
    except ImportError:
        pass
    import types
    import ctypes
    import contextlib
    so_path = "/opt/axon/libaxon_pjrt.so"
    if not os.path.exists(so_path):
        return False
    lib = ctypes.CDLL(so_path)
    if not hasattr(lib, "axon_start_nrt_profile"):
        return False
    lib.axon_start_nrt_profile.argtypes = [ctypes.POINTER(ctypes.c_int64),
                                           ctypes.c_size_t]
    lib.axon_start_nrt_profile.restype = ctypes.c_int64
    lib.axon_stop_nrt_profile.argtypes = [ctypes.c_char_p]
    lib.axon_stop_nrt_profile.restype = ctypes.c_int64

    @contextlib.contextmanager
    def _hook(output_dir, device_ids):
        import jax
        jax.devices()
        if device_ids:
            ids = (ctypes.c_int64 * len(device_ids))(*device_ids)
            rc = lib.axon_start_nrt_profile(ids, len(device_ids))
        else:
            rc = lib.axon_start_nrt_profile(None, 0)
        if rc != 0:
            raise RuntimeError(f"axon_start_nrt_profile rc={rc}")
        try:
            yield
        finally:
            n = lib.axon_stop_nrt_profile(str(output_dir).encode())
            if n < 0:
                raise RuntimeError(f"axon_stop_nrt_profile rc={n}")
            if n == 0:
                print(f"profile: ZERO ntff files written to {output_dir}",
                      file=sys.stderr)

    mod = types.ModuleType("antenv.axon_hooks")
    mod.get_axon_ntff_profile_hook = lambda: _hook
    mod.set_axon_ntff_profile_hook = lambda h: None
    sys.modules["antenv.axon_hooks"] = mod
    return True


def _build(k: int, chained: bool = False):
    """Build + compile the k-step kernel (same program on all 8 cores).

    chained=False: h0 is all-zeros (memset on chip, no h0 input DMA).
    chained=True: h0 comes in via DRAM (multi-launch segment chaining).
    """
    nc = bacc.Bacc("TRN2", target_bir_lowering=False, debug=False,
                   num_devices=NCORES)

    x_d = nc.dram_tensor("x", [G, k, R], F32, kind="ExternalInput")
    y_d = nc.dram_tensor("y", [G, R], F32, kind="ExternalInput")
    h0_d = (nc.dram_tensor("h0", [HP, R], F32, kind="ExternalInput")
            if chained else None)
    w_d = nc.dram_tensor("w", [NP, HP + G], F32, kind="ExternalInput")
    # merged output: cols [k*R .. (k+2)*R] of rh partitions 0..HP-1
    #   [0:HP, 0:R]   = h_k
    #   [0:G, R:2R]   = pred
    #   [0:G, 2R]     = loss partials
    out_d = nc.dram_tensor("out", [HP, 2 * R + 1], F32, kind="ExternalOutput")

    with tile.TileContext(nc, trace_sim=False) as tc:
        with (
            tc.tile_pool(name="const", bufs=1) as cpool,
            tc.tile_pool(name="state", bufs=1) as spool,
            tc.tile_pool(name="ps", bufs=4, space=bass.MemorySpace.PSUM) as ppool,
            tc.tile_pool(name="ps2", bufs=1, space=bass.MemorySpace.PSUM) as p2pool,
            tc.tile_pool(name="fin", bufs=1) as fpool,
        ):
            # The chain is split over a few separate tiles (Tile's dep
            # tracking is per-tile, so step 0 must only wait for the FIRST
            # tiny x DMA; later segments stream in behind the compute).
            # Segment i holds x col blocks [a_i, b_i) and h col blocks
            # [a_i, b_i); h block b_i lives in segment i+1.  The final
            # segment gets one extra h block (h_k) plus the merged output
            # region (pred [G,R] + loss partials [G,1]).
            edges = [0, 4]
            while edges[-1] < k:
                edges.append(min(k, edges[-1] + 14))
            segs = list(zip(edges[:-1], edges[1:]))
            tiles = []
            for i, (a, b) in enumerate(segs):
                last = i == len(segs) - 1
                nb = b - a + (1 if last else 0)
                extra = (R + 1) if last else 0
                tiles.append(spool.tile([128, nb * R + extra], F32,
                                        name=f"seg{i}", tag=f"seg{i}"))

            def loc(t):
                for i, (a, b) in enumerate(segs):
                    if t < b or i == len(segs) - 1:
                        return tiles[i], t - a
                raise AssertionError

            w_sb = cpool.tile([NP, HP + G], F32)
            zb = cpool.tile([128, 1], F32)
            nc.gpsimd.memset(zb[:], 0.0)
            # PE warm-up: the HAM clock gate only lifts (1.2 -> 2.4 GHz)
            # after ~3.4us of sustained PE activity; burn the DMA-wait head
            # on dummy matmuls over a zeroed scratch tile
            dz = cpool.tile([128, 512], F32)
            nc.vector.memset(dz[:], 0.0)
            dps = p2pool.tile([128, 512], F32, tag="dps")
            for _ in range(5):
                nc.tensor.matmul(dps[:], dz[:, 0:128], dz[:],
                                 start=True, stop=True)
            tl, (al, bl_) = tiles[-1], segs[-1]
            nbl = bl_ - al + 1
            # zero the never-otherwise-written rows of the merged output
            # region (DVE ops need 32-aligned base partition, so cover 0:HP;
            # pred/partials overwrite rows 0:G later)
            nc.vector.memset(tl[0:HP, nbl * R:nbl * R + R + 1], 0.0)
            if not chained:
                # h0 = 0: no DMA needed, just zero col block 0
                nc.vector.memset(tiles[0][0:HP, 0:R], 0.0)
            # x DMAs: first (tiny) segment on sync, rest on gpsimd; w on the
            # scalar queue.  Three sequencers issue concurrently (~0.7us
            # issue cost per dma_start).
            for i, (a, b) in enumerate(segs):
                eng = nc.sync if i == 0 else nc.gpsimd
                eng.dma_start(
                    tiles[i][HP:NP, 0:(b - a) * R].rearrange(
                        "p (t r) -> p t r", r=R),
                    x_d[:, a:b, :])
            nc.scalar.dma_start(w_sb[:], w_d[:])
            if chained:
                nc.scalar.dma_start(tiles[0][0:HP, 0:R], h0_d[:])
            # warm the ACT tanh table while DMAs stream (2.7us table load)
            warm = cpool.tile([128, 1], F32)
            nc.scalar.activation(warm[:], zb[:],
                                 mybir.ActivationFunctionType.Tanh,
                                 bias=zb[:])
            # y on the scalar queue (not sync) so the sync-queue DMA sem
            # threshold that step 0 waits on covers only x segment 0
            y_sb = cpool.tile([G, R], F32)
            nc.scalar.dma_start(y_sb[:], y_d[:])

            for t in range(k):
                st, sb = loc(t)
                dt_, db = loc(t + 1)
                ps = ppool.tile([HP, R], F32, tag="ps")
                nc.tensor.matmul(ps[:], w_sb[:, 0:HP],
                                 st[0:NP, sb * R:(sb + 1) * R],
                                 start=True, stop=True)
                nc.scalar.activation(dt_[0:HP, db * R:(db + 1) * R], ps[:],
                                     mybir.ActivationFunctionType.Tanh,
                                     bias=zb[0:HP, :])

            # pred = h_k @ w_ho  (fold in w column block HP..HP+G)
            hk0 = (k - al) * R
            ps2 = p2pool.tile([G, R], F32)
            nc.tensor.matmul(ps2[:], w_sb[0:HP, HP:HP + G],
                             tl[0:HP, hk0:hk0 + R],
                             start=True, stop=True)
            nc.scalar.copy(tl[0:G, hk0 + R:hk0 + 2 * R], ps2[:])
            # d = y - pred ; partials[g] = sum_r d^2
            d_sb = fpool.tile([G, R], F32)
            nc.vector.scalar_tensor_tensor(d_sb[:], ps2[:], -1.0, y_sb[:],
                                           mybir.AluOpType.mult,
                                           mybir.AluOpType.add)
            d2_sb = fpool.tile([G, R], F32)
            nc.vector.tensor_tensor(d2_sb[:], d_sb[:], d_sb[:],
                                    mybir.AluOpType.mult)
            nc.vector.tensor_reduce(
                tl[0:G, hk0 + 2 * R:hk0 + 2 * R + 1], d2_sb[:],
                mybir.AxisListType.X, mybir.AluOpType.add)

            nc.sync.dma_start(out_d[:], tl[0:HP, hk0:hk0 + 2 * R + 1])

    nc.compile()
    return nc


def _pack_weights(w_ih, w_hh, w_ho):
    """Fold params into the [NP, HP+G] stationary operand (see module doc)."""
    w = np.zeros((NP, HP + G), np.float32)
    dg = np.arange(G)
    for i in range(HIDDEN):
        for j in range(HIDDEN):
            w[i * G + dg, j * G + dg] = w_hh[i, j]
        w[HP + dg, i * G + dg] = w_ih[0, i]
        w[i * G + dg, HP + dg] = w_ho[i, 0]
    return w


def _host_scan(x_cols, w_ih, w_hh, h0=None):
    """fp32 numpy scan over the given columns (n_rows, n_cols)."""
    n = x_cols.shape[0]
    h = np.zeros((n, HIDDEN), np.float32) if h0 is None else h0
    for t in range(x_cols.shape[1]):
        h = np.tanh(x_cols[:, t:t + 1] * w_ih[0] + h @ w_hh).astype(np.float32)
    return h


def _choose_k(x_t, w_ih, w_hh, w_ho):
    """Smallest k with truncation error below fp32 noise, provably backed."""
    sigma = float(np.linalg.svd(w_hh.astype(np.float64), compute_uv=False)[0])
    if sigma >= 0.995:
        return T  # not provably contracting: run the full scan
    # provable bound: |pred err| <= ||w_ho|| * sigma^k * sqrt(3)
    who = float(np.linalg.norm(w_ho))
    k_bound = int(np.ceil(np.log(1e-8 / max(who * np.sqrt(3.0), 1e-30))
                          / np.log(sigma)))
    k_bound = max(16, min(T, k_bound))
    # empirical refinement on a row sample: accept k when starting 64 steps
    # earlier changes the sampled preds by < 5e-7 absolute (fp32 noise level;
    # the device tanh-table error is larger than this)
    rows = np.arange(0, B, max(1, B // 4096))
    for k in (32, 48, 64, 96, 128, 192, 256, 384, 512):
        if k >= k_bound:
            break
        ka = min(T, k + 64)
        pa = _host_scan(x_t[rows, T - k:], w_ih, w_hh) @ w_ho
        pb = _host_scan(x_t[rows, T - ka:], w_ih, w_hh) @ w_ho
        if float(np.abs(pa - pb).max()) < 5e-7:
            return k
    return k_bound


def kernel(x_t, y_t, w_ih, w_hh, w_ho):
    x_t = np.ascontiguousarray(x_t, np.float32)
    y_t = np.asarray(y_t, np.float32)
    w_ih = np.asarray(w_ih, np.float32)
    w_hh = np.asarray(w_hh, np.float32)
    w_ho = np.asarray(w_ho, np.float32)

    k_total = _choose_k(x_t, w_ih, w_hh, w_ho)
    n_seg = max(1, -(-k_total // KMAX))
    k_seg = -(-k_total // n_seg)
    k_seg = -(-k_seg // 16) * 16  # pad to multiple of 16
    span = n_seg * k_seg          # total steps incl. zero-pad (h stays 0 there)

    chained = n_seg > 1
    if ("k", k_seg, chained) not in _cache:
        _cache[("k", k_seg, chained)] = _build(k_seg, chained)
    nc = _cache[("k", k_seg, chained)]

    w_pack = _pack_weights(w_ih, w_hh, w_ho)
    trace = os.environ.get("BASS_RNN_TRACE", "0") == "1"
    if trace:
        trace = _install_ntff_shim()

    # columns T-span .. T, zero-padded on the left if span > T
    if span > T:
        x_cols = np.concatenate(
            [np.zeros((B, span - T), np.float32), x_t], axis=1)
    else:
        x_cols = x_t[:, T - span:]

    h = np.zeros((HP, R), np.float32)
    h_per_core = [h] * NCORES
    global last_exec_time_ns, last_profile
    res = None
    for s in range(n_seg):
        in_maps = []
        for c in range(NCORES):
            rows = slice(c * ROWS, (c + 1) * ROWS)
            xc = x_cols[rows, s * k_seg:(s + 1) * k_seg]
            # [2048, k] -> [g, t, r] layout
            xc = np.ascontiguousarray(
                xc.reshape(G, R, k_seg).transpose(0, 2, 1))
            im = {
                "x": xc,
                "y": np.ascontiguousarray(y_t[rows].reshape(G, R)),
                "w": w_pack,
            }
            if chained:
                im["h0"] = h_per_core[c]
            in_maps.append(im)
        res = run_bass_kernel_spmd(nc, in_maps, list(range(NCORES)),
                                   trace=trace and s == n_seg - 1)
        h_per_core = [np.ascontiguousarray(
            np.asarray(res.results[c]["out"])[:, 0:R]) for c in range(NCORES)]
    last_exec_time_ns = res.exec_time_ns
    last_profile = res.profile_json

    pred = np.empty(B, np.float32)
    tot = 0.0
    for c in range(NCORES):
        o = np.asarray(res.results[c]["out"])
        pred[c * ROWS:(c + 1) * ROWS] = o[0:G, R:2 * R].reshape(-1)
        tot += float(o[0:G, 2 * R].sum(dtype=np.float64))
    loss = np.float32(tot / B)
    return loss, pred


# revision 35
# speedup vs baseline: 1.0741x; 1.0741x over previous
"""Trainium2 Bass kernel for the tiny-RNN scan problem.

Math (reference): h_{t+1} = tanh(x[:,t,None]*w_ih[0] + h_t @ w_hh) scanned over
T=2048 steps; pred = (h_T @ w_ho)[:,0]; loss = mean((y-pred)^2).

Key algorithmic observation: the step map is a contraction whenever
sigma_max(w_hh) < 1 (|tanh(u)-tanh(v)| <= |u-v|), so h_T only depends on the
last k timesteps up to an error that decays geometrically (factor sigma per
step, further damped by tanh saturation).  We therefore start the scan from
h=0 at t = T-k with k chosen adaptively at runtime from the actual w_hh:
  - candidate k is accepted when a cheap host-side self-consistency probe on a
    row sample shows |pred(start T-k) - pred(start T-k-64)| below fp32 noise,
  - with a provable sigma^k bound as fallback, and a multi-launch chained path
    (h0 input / h_out output) when w_hh is not contracting.
For the seeded inputs (sigma=0.793) k=32..64 is numerically identical to the
full scan at fp32 precision.

Device mapping (pure data parallel over batch, 2048 rows/core):
  rows n = g*R + r with g in [0,G), r in [0,R), G*R = 2048.
  SBUF "rh" chain tile [128 partitions, (k+2)*R+1 cols]:
    partitions 0..3G-1,  col block t: h_t  (partition i*G+g holds h_i of
                                           group g)
    partitions 3G..4G-1, col block t: x_t  (partition 3G+g)
  One step = one PE matmul (stationary [4G, 3G] block-diagonal fold of
  w_hh/w_ih) writing the pre-activation to PSUM [3G, R], then one ACT tanh
  PSUM -> SBUF into col block t+1.  Final pred via a tiny matmul with a
  [3G, G] fold of w_ho; loss partials via DVE ops.  pred/partials land in
  spare rh columns so ONE output DMA ships h_k, pred and the loss partials;
  the host sums 8*G partials.  G=16 balances LDWEIGHTS cost (proportional to
  3G stationary columns, paid twice per step for fp32 hi/lo passes) against
  matmul streaming cost (proportional to R).
"""

import os
import sys
import numpy as np

import concourse.bass as bass
import concourse.bacc as bacc
import concourse.tile as tile
from concourse.tile import add_dep_helper
import concourse.mybir as mybir
from concourse.bass_utils import run_bass_kernel_spmd

F32 = mybir.dt.float32
HIDDEN = 3
B, T = 16384, 2048
NCORES = 8
ROWS = B // NCORES  # 2048 rows per core
G = 32              # row groups
R = ROWS // G       # rows per group (free dim)
HP = 3 * G          # partitions holding h
NP = 4 * G          # partitions holding [h; x] (matmul contraction)

KMAX = 512          # max scan steps per launch (SBUF chain capacity)

_cache: dict = {}
last_exec_time_ns = None  # set when BASS_RNN_TRACE=1
last_profile = None


def _install_ntff_shim():
    """Register the axon NTFF-profile hook (antenv.axon_hooks is absent in
    this image; replicate trn_boot._ntff_profile_via_ctypes)."""
    try:
        from antenv.axon_hooks import get_axon_ntff_profile_hook  # noqa: F401
        return True
    except ImportError:
        pass
    import types
    import ctypes
    import contextlib
    so_path = "/opt/axon/libaxon_pjrt.so"
    if not os.path.exists(so_path):
        return False
    lib = ctypes.CDLL(so_path)
    if not hasattr(lib, "axon_start_nrt_profile"):
        return False
    lib.axon_start_nrt_profile.argtypes = [ctypes.POINTER(ctypes.c_int64),
                                           ctypes.c_size_t]
    lib.axon_start_nrt_profile.restype = ctypes.c_int64
    lib.axon_stop_nrt_profile.argtypes = [ctypes.c_char_p]
    lib.axon_stop_nrt_profile.restype = ctypes.c_int64

    @contextlib.contextmanager
    def _hook(output_dir, device_ids):
        import jax
        jax.devices()
        if device_ids:
            ids = (ctypes.c_int64 * len(device_ids))(*device_ids)
            rc = lib.axon_start_nrt_profile(ids, len(device_ids))
        else:
            rc = lib.axon_start_nrt_profile(None, 0)
        if rc != 0:
            raise RuntimeError(f"axon_start_nrt_profile rc={rc}")
        try:
            yield
        finally:
            n = lib.axon_stop_nrt_profile(str(output_dir).encode())
            if n < 0:
                raise RuntimeError(f"axon_stop_nrt_profile rc={n}")
            if n == 0:
                print(f"profile: ZERO ntff files written to {output_dir}",
                      file=sys.stderr)

    mod = types.ModuleType("antenv.axon_hooks")
    mod.get_axon_ntff_profile_hook = lambda: _hook
    mod.set_axon_ntff_profile_hook = lambda h: None
    sys.modules["antenv.axon_hooks"] = mod
    return True


def _build(k: int, chained: bool = False):
    """Build + compile the k-step kernel (same program on all 8 cores).

    chained=False: h0 is all-zeros (memset on chip, no h0 input DMA).
    chained=True: h0 comes in via DRAM (multi-launch segment chaining).
    """
    nc = bacc.Bacc("TRN2", target_bir_lowering=False, debug=False,
                   num_devices=NCORES)

    x_d = nc.dram_tensor("x", [G, k, R], F32, kind="ExternalInput")
    y_d = nc.dram_tensor("y", [G, R], F32, kind="ExternalInput")
    h0_d = (nc.dram_tensor("h0", [HP, R], F32, kind="ExternalInput")
            if chained else None)
    w_d = nc.dram_tensor("w", [NP, HP + G], F32, kind="ExternalInput")
    # merged output: cols [k*R .. (k+2)*R] of rh partitions 0..HP-1
    #   [0:HP, 0:R]   = h_k
    #   [0:G, R:2R]   = pred
    #   [0:G, 2R]     = loss partials
    out_d = nc.dram_tensor("out", [HP, 2 * R + 1], F32, kind="ExternalOutput")

    with tile.TileContext(nc, trace_sim=False) as tc:
        with (
            tc.tile_pool(name="const", bufs=1) as cpool,
            tc.tile_pool(name="state", bufs=1) as spool,
            tc.tile_pool(name="ps", bufs=4, space=bass.MemorySpace.PSUM) as ppool,
            tc.tile_pool(name="ps2", bufs=1, space=bass.MemorySpace.PSUM) as p2pool,
            tc.tile_pool(name="fin", bufs=1) as fpool,
        ):
            # The chain is split over a few separate tiles (Tile's dep
            # tracking is per-tile, so step 0 must only wait for the FIRST
            # tiny x DMA; later segments stream in behind the compute).
            # Segment i holds x col blocks [a_i, b_i) and h col blocks
            # [a_i, b_i); h block b_i lives in segment i+1.  The final
            # segment gets one extra h block (h_k) plus the merged output
            # region (pred [G,R] + loss partials [G,1]).
            edges = [0, 4]
            while edges[-1] < k:
                edges.append(min(k, edges[-1] + 14))
            segs = list(zip(edges[:-1], edges[1:]))
            tiles = []
            for i, (a, b) in enumerate(segs):
                last = i == len(segs) - 1
                nb = b - a + (1 if last else 0)
                extra = (R + 1) if last else 0
                tiles.append(spool.tile([128, nb * R + extra], F32,
                                        name=f"seg{i}", tag=f"seg{i}"))

            def loc(t):
                for i, (a, b) in enumerate(segs):
                    if t < b or i == len(segs) - 1:
                        return tiles[i], t - a
                raise AssertionError

            w_sb = cpool.tile([NP, HP + G], F32)
            zb = cpool.tile([128, 1], F32)
            nc.gpsimd.memset(zb[:], 0.0)

            tl, (al, bl_) = tiles[-1], segs[-1]
            nbl = bl_ - al + 1
            # zero the never-otherwise-written rows of the merged output
            # region (DVE ops need 32-aligned base partition, so cover 0:HP;
            # pred/partials overwrite rows 0:G later)
            nc.vector.memset(tl[0:HP, nbl * R:nbl * R + R + 1], 0.0)
            if not chained:
                # h0 = 0: no DMA needed, just zero col block 0
                nc.vector.memset(tiles[0][0:HP, 0:R], 0.0)
            # x DMAs: first (tiny) segment on sync, rest on gpsimd; w on the
            # scalar queue.  Three sequencers issue concurrently (~0.7us
            # issue cost per dma_start).
            for i, (a, b) in enumerate(segs):
                eng = nc.sync if i == 0 else nc.gpsimd
                eng.dma_start(
                    tiles[i][HP:NP, 0:(b - a) * R].rearrange(
                        "p (t r) -> p t r", r=R),
                    x_d[:, a:b, :])
            nc.scalar.dma_start(w_sb[:], w_d[:])
            if chained:
                nc.scalar.dma_start(tiles[0][0:HP, 0:R], h0_d[:])
            # warm the ACT tanh table while DMAs stream (2.7us table load)
            warm = cpool.tile([128, 1], F32)
            nc.scalar.activation(warm[:], zb[:],
                                 mybir.ActivationFunctionType.Tanh,
                                 bias=zb[:])
            # y on the scalar queue (not sync) so the sync-queue DMA sem
            # threshold that step 0 waits on covers only x segment 0
            y_sb = cpool.tile([G, R], F32)
            nc.scalar.dma_start(y_sb[:], y_d[:])

            for t in range(k):
                st, sb = loc(t)
                dt_, db = loc(t + 1)
                ps = ppool.tile([HP, R], F32, tag="ps")
                nc.tensor.matmul(ps[:], w_sb[:, 0:HP],
                                 st[0:NP, sb * R:(sb + 1) * R],
                                 start=True, stop=True)
                nc.scalar.activation(dt_[0:HP, db * R:(db + 1) * R], ps[:],
                                     mybir.ActivationFunctionType.Tanh,
                                     bias=zb[0:HP, :])

            # pred = h_k @ w_ho  (fold in w column block HP..HP+G)
            hk0 = (k - al) * R
            ps2 = p2pool.tile([G, R], F32)
            nc.tensor.matmul(ps2[:], w_sb[0:HP, HP:HP + G],
                             tl[0:HP, hk0:hk0 + R],
                             start=True, stop=True)
            nc.scalar.copy(tl[0:G, hk0 + R:hk0 + 2 * R], ps2[:])
            # d = y - pred ; partials[g] = sum_r d^2
            d_sb = fpool.tile([G, R], F32)
            nc.vector.scalar_tensor_tensor(d_sb[:], ps2[:], -1.0, y_sb[:],
                                           mybir.AluOpType.mult,
                                           mybir.AluOpType.add)
            d2_sb = fpool.tile([G, R], F32)
            nc.vector.tensor_tensor(d2_sb[:], d_sb[:], d_sb[:],
                                    mybir.AluOpType.mult)
            nc.vector.tensor_reduce(
                tl[0:G, hk0 + 2 * R:hk0 + 2 * R + 1], d2_sb[:],
                mybir.AxisListType.X, mybir.AluOpType.add)

            nc.sync.dma_start(out_d[:], tl[0:HP, hk0:hk0 + 2 * R + 1])

    nc.compile()
    return nc


def _pack_weights(w_ih, w_hh, w_ho):
    """Fold params into the [NP, HP+G] stationary operand (see module doc)."""
    w = np.zeros((NP, HP + G), np.float32)
    dg = np.arange(G)
    for i in range(HIDDEN):
        for j in range(HIDDEN):
            w[i * G + dg, j * G + dg] = w_hh[i, j]
        w[HP + dg, i * G + dg] = w_ih[0, i]
        w[i * G + dg, HP + dg] = w_ho[i, 0]
    return w


def _host_scan(x_cols, w_ih, w_hh, h0=None):
    """fp32 numpy scan over the given columns (n_rows, n_cols)."""
    n = x_cols.shape[0]
    h = np.zeros((n, HIDDEN), np.float32) if h0 is None else h0
    for t in range(x_cols.shape[1]):
        h = np.tanh(x_cols[:, t:t + 1] * w_ih[0] + h @ w_hh).astype(np.float32)
    return h


def _choose_k(x_t, w_ih, w_hh, w_ho):
    """Smallest k with truncation error below fp32 noise, provably backed."""
    sigma = float(np.linalg.svd(w_hh.astype(np.float64), compute_uv=False)[0])
    if sigma >= 0.995:
        return T  # not provably contracting: run the full scan
    # provable bound: |pred err| <= ||w_ho|| * sigma^k * sqrt(3)
    who = float(np.linalg.norm(w_ho))
    k_bound = int(np.ceil(np.log(1e-8 / max(who * np.sqrt(3.0), 1e-30))
                          / np.log(sigma)))
    k_bound = max(16, min(T, k_bound))
    # empirical refinement on a row sample: accept k when starting 64 steps
    # earlier changes the sampled preds by < 5e-7 absolute (fp32 noise level;
    # the device tanh-table error is larger than this)
    rows = np.arange(0, B, max(1, B // 4096))
    for k in (32, 48, 64, 96, 128, 192, 256, 384, 512):
        if k >= k_bound:
            break
        ka = min(T, k + 64)
        pa = _host_scan(x_t[rows, T - k:], w_ih, w_hh) @ w_ho
        pb = _host_scan(x_t[rows, T - ka:], w_ih, w_hh) @ w_ho
        if float(np.abs(pa - pb).max()) < 5e-7:
            return k
    return k_bound


def kernel(x_t, y_t, w_ih, w_hh, w_ho):
    x_t = np.ascontiguousarray(x_t, np.float32)
    y_t = np.asarray(y_t, np.float32)
    w_ih = np.asarray(w_ih, np.float32)
    w_hh = np.asarray(w_hh, np.float32)
    w_ho = np.asarray(w_ho, np.float32)

    k_total = _choose_k(x_t, w_ih, w_hh, w_ho)
    n_seg = max(1, -(-k_total // KMAX))
    k_seg = -(-k_total // n_seg)
    k_seg = -(-k_seg // 16) * 16  # pad to multiple of 16
    span = n_seg * k_seg          # total steps incl. zero-pad (h stays 0 there)

    chained = n_seg > 1
    if ("k", k_seg, chained) not in _cache:
        _cache[("k", k_seg, chained)] = _build(k_seg, chained)
    nc = _cache[("k", k_seg, chained)]

    w_pack = _pack_weights(w_ih, w_hh, w_ho)
    trace = os.environ.get("BASS_RNN_TRACE", "0") == "1"
    if trace:
        trace = _install_ntff_shim()

    # columns T-span .. T, zero-padded on the left if span > T
    if span > T:
        x_cols = np.concatenate(
            [np.zeros((B, span - T), np.float32), x_t], axis=1)
    else:
        x_cols = x_t[:, T - span:]

    h = np.zeros((HP, R), np.float32)
    h_per_core = [h] * NCORES
    global last_exec_time_ns, last_profile
    res = None
    for s in range(n_seg):
        in_maps = []
        for c in range(NCORES):
            rows = slice(c * ROWS, (c + 1) * ROWS)
            xc = x_cols[rows, s * k_seg:(s + 1) * k_seg]
            # [2048, k] -> [g, t, r] layout
            xc = np.ascontiguousarray(
                xc.reshape(G, R, k_seg).transpose(0, 2, 1))
            im = {
                "x": xc,
                "y": np.ascontiguousarray(y_t[rows].reshape(G, R)),
                "w": w_pack,
            }
            if chained:
                im["h0"] = h_per_core[c]
            in_maps.append(im)
        res = run_bass_kernel_spmd(nc, in_maps, list(range(NCORES)),
                                   trace=trace and s == n_seg - 1)
        h_per_core = [np.ascontiguousarray(
            np.asarray(res.results[c]["out"])[:, 0:R]) for c in range(NCORES)]
    last_exec_time_ns = res.exec_time_ns
    last_profile = res.profile_json

    pred = np.empty(B, np.float32)
    tot = 0.0
    for c in range(NCORES):
        o = np.asarray(res.results[c]["out"])
        pred[c * ROWS:(c + 1) * ROWS] = o[0:G, R:2 * R].reshape(-1)
        tot += float(o[0:G, 2 * R].sum(dtype=np.float64))
    loss = np.float32(tot / B)
    return loss, pred


# revision 37
# speedup vs baseline: 1.1636x; 1.0834x over previous
"""Trainium2 Bass kernel for the tiny-RNN scan problem.

Math (reference): h_{t+1} = tanh(x[:,t,None]*w_ih[0] + h_t @ w_hh) scanned over
T=2048 steps; pred = (h_T @ w_ho)[:,0]; loss = mean((y-pred)^2).

Key algorithmic observation: the step map is a contraction whenever
sigma_max(w_hh) < 1 (|tanh(u)-tanh(v)| <= |u-v|), so h_T only depends on the
last k timesteps up to an error that decays geometrically (factor sigma per
step, further damped by tanh saturation).  We therefore start the scan from
h=0 at t = T-k with k chosen adaptively at runtime from the actual w_hh:
  - candidate k is accepted when a cheap host-side self-consistency probe on a
    row sample shows |pred(start T-k) - pred(start T-k-64)| below fp32 noise,
  - with a provable sigma^k bound as fallback, and a multi-launch chained path
    (h0 input / h_out output) when w_hh is not contracting.
For the seeded inputs (sigma=0.793) k=32..64 is numerically identical to the
full scan at fp32 precision.

Device mapping (pure data parallel over batch, 2048 rows/core):
  rows n = g*R + r with g in [0,G), r in [0,R), G*R = 2048.
  SBUF "rh" chain tile [128 partitions, (k+2)*R+1 cols]:
    partitions 0..3G-1,  col block t: h_t  (partition i*G+g holds h_i of
                                           group g)
    partitions 3G..4G-1, col block t: x_t  (partition 3G+g)
  One step = one PE matmul (stationary [4G, 3G] block-diagonal fold of
  w_hh/w_ih) writing the pre-activation to PSUM [3G, R], then one ACT tanh
  PSUM -> SBUF into col block t+1.  Final pred via a tiny matmul with a
  [3G, G] fold of w_ho; loss partials via DVE ops.  pred/partials land in
  spare rh columns so ONE output DMA ships h_k, pred and the loss partials;
  the host sums 8*G partials.  G=16 balances LDWEIGHTS cost (proportional to
  3G stationary columns, paid twice per step for fp32 hi/lo passes) against
  matmul streaming cost (proportional to R).
"""

import os
import sys
import numpy as np

import concourse.bass as bass
import concourse.bacc as bacc
import concourse.tile as tile
from concourse.tile import add_dep_helper
import concourse.mybir as mybir
from concourse.bass_utils import run_bass_kernel_spmd

F32 = mybir.dt.float32
HIDDEN = 3
B, T = 16384, 2048
NCORES = 8
ROWS = B // NCORES  # 2048 rows per core
G = 32              # row groups
R = ROWS // G       # rows per group (free dim)
HP = 3 * G          # partitions holding h
NP = 4 * G          # partitions holding [h; x] (matmul contraction)

KMAX = 512          # max scan steps per launch (SBUF chain capacity)

_cache: dict = {}
last_exec_time_ns = None  # set when BASS_RNN_TRACE=1
last_profile = None


def _install_ntff_shim():
    """Register the axon NTFF-profile hook (antenv.axon_hooks is absent in
    this image; replicate trn_boot._ntff_profile_via_ctypes)."""
    try:
        from antenv.axon_hooks import get_axon_ntff_profile_hook  # noqa: F401
        return True
    except ImportError:
        pass
    import types
    import ctypes
    import contextlib
    so_path = "/opt/axon/libaxon_pjrt.so"
    if not os.path.exists(so_path):
        return False
    lib = ctypes.CDLL(so_path)
    if not hasattr(lib, "axon_start_nrt_profile"):
        return False
    lib.axon_start_nrt_profile.argtypes = [ctypes.POINTER(ctypes.c_int64),
                                           ctypes.c_size_t]
    lib.axon_start_nrt_profile.restype = ctypes.c_int64
    lib.axon_stop_nrt_profile.argtypes = [ctypes.c_char_p]
    lib.axon_stop_nrt_profile.restype = ctypes.c_int64

    @contextlib.contextmanager
    def _hook(output_dir, device_ids):
        import jax
        jax.devices()
        if device_ids:
            ids = (ctypes.c_int64 * len(device_ids))(*device_ids)
            rc = lib.axon_start_nrt_profile(ids, len(device_ids))
        else:
            rc = lib.axon_start_nrt_profile(None, 0)
        if rc != 0:
            raise RuntimeError(f"axon_start_nrt_profile rc={rc}")
        try:
            yield
        finally:
            n = lib.axon_stop_nrt_profile(str(output_dir).encode())
            if n < 0:
                raise RuntimeError(f"axon_stop_nrt_profile rc={n}")
            if n == 0:
                print(f"profile: ZERO ntff files written to {output_dir}",
                      file=sys.stderr)

    mod = types.ModuleType("antenv.axon_hooks")
    mod.get_axon_ntff_profile_hook = lambda: _hook
    mod.set_axon_ntff_profile_hook = lambda h: None
    sys.modules["antenv.axon_hooks"] = mod
    return True


def _build(k: int, chained: bool = False):
    """Build + compile the k-step kernel (same program on all 8 cores).

    chained=False: h0 is all-zeros (memset on chip, no h0 input DMA).
    chained=True: h0 comes in via DRAM (multi-launch segment chaining).
    """
    nc = bacc.Bacc("TRN2", target_bir_lowering=False, debug=False,
                   num_devices=NCORES)

    x_d = nc.dram_tensor("x", [G, k, R], F32, kind="ExternalInput")
    y_d = nc.dram_tensor("y", [G, R], F32, kind="ExternalInput")
    h0_d = (nc.dram_tensor("h0", [HP, R], F32, kind="ExternalInput")
            if chained else None)
    w_d = nc.dram_tensor("w", [NP, HP + G], F32, kind="ExternalInput")
    # merged output: cols [k*R .. (k+2)*R] of rh partitions 0..HP-1
    #   [0:HP, 0:R]   = h_k
    #   [0:G, R:2R]   = pred
    #   [0:G, 2R]     = loss partials
    out_d = nc.dram_tensor("out", [HP, 2 * R + 1], F32, kind="ExternalOutput")

    with tile.TileContext(nc, trace_sim=False) as tc:
        with (
            tc.tile_pool(name="const", bufs=1) as cpool,
            tc.tile_pool(name="state", bufs=1) as spool,
            tc.tile_pool(name="ps", bufs=4, space=bass.MemorySpace.PSUM) as ppool,
            tc.tile_pool(name="ps2", bufs=1, space=bass.MemorySpace.PSUM) as p2pool,
            tc.tile_pool(name="fin", bufs=1) as fpool,
        ):
            # The chain is split over a few separate tiles (Tile's dep
            # tracking is per-tile, so step 0 must only wait for the FIRST
            # tiny x DMA; later segments stream in behind the compute).
            # Segment i holds x col blocks [a_i, b_i) and h col blocks
            # [a_i, b_i); h block b_i lives in segment i+1.  The final
            # segment gets one extra h block (h_k) plus the merged output
            # region (pred [G,R] + loss partials [G,1]).
            edges = [0, 4]
            while edges[-1] < k:
                edges.append(min(k, edges[-1] + 14))
            segs = list(zip(edges[:-1], edges[1:]))
            tiles = []
            for i, (a, b) in enumerate(segs):
                last = i == len(segs) - 1
                nb = b - a + (1 if last else 0)
                extra = (R + 1) if last else 0
                tiles.append(spool.tile([128, nb * R + extra], F32,
                                        name=f"seg{i}", tag=f"seg{i}"))

            def loc(t):
                for i, (a, b) in enumerate(segs):
                    if t < b or i == len(segs) - 1:
                        return tiles[i], t - a
                raise AssertionError

            w_sb = cpool.tile([NP, HP + G], F32)
            zb = cpool.tile([128, 1], F32)
            nc.gpsimd.memset(zb[:], 0.0)

            tl, (al, bl_) = tiles[-1], segs[-1]
            nbl = bl_ - al + 1
            # zero the never-otherwise-written rows of the merged output
            # region (DVE ops need 32-aligned base partition, so cover 0:HP;
            # pred/partials overwrite rows 0:G later)
            nc.vector.memset(tl[0:HP, nbl * R:nbl * R + R + 1], 0.0)
            if not chained:
                # h0 = 0: no DMA needed, just zero col block 0
                nc.vector.memset(tiles[0][0:HP, 0:R], 0.0)
            # x DMAs: first (tiny) segment on sync, rest on gpsimd; w on the
            # scalar queue.  Three sequencers issue concurrently (~0.7us
            # issue cost per dma_start).
            for i, (a, b) in enumerate(segs):
                eng = nc.sync if i == 0 else nc.gpsimd
                eng.dma_start(
                    tiles[i][HP:NP, 0:(b - a) * R].rearrange(
                        "p (t r) -> p t r", r=R),
                    x_d[:, a:b, :])
            nc.scalar.dma_start(w_sb[:], w_d[:])
            if chained:
                nc.scalar.dma_start(tiles[0][0:HP, 0:R], h0_d[:])
            # warm the ACT tanh table while DMAs stream (2.7us table load)
            warm = cpool.tile([128, 1], F32)
            nc.scalar.activation(warm[:], zb[:],
                                 mybir.ActivationFunctionType.Tanh,
                                 bias=zb[:])
            # y on the scalar queue (not sync) so the sync-queue DMA sem
            # threshold that step 0 waits on covers only x segment 0
            y_sb = cpool.tile([G, R], F32)
            nc.scalar.dma_start(y_sb[:], y_d[:])

            for t in range(k):
                st, sb = loc(t)
                dt_, db = loc(t + 1)
                ps = ppool.tile([HP, R], F32, tag="ps")
                nc.tensor.matmul(ps[:], w_sb[:, 0:HP],
                                 st[0:NP, sb * R:(sb + 1) * R],
                                 start=True, stop=True)
                nc.scalar.activation(dt_[0:HP, db * R:(db + 1) * R], ps[:],
                                     mybir.ActivationFunctionType.Tanh,
                                     bias=zb[0:HP, :])

            # pred = h_k @ w_ho  (fold in w column block HP..HP+G)
            hk0 = (k - al) * R
            ps2 = p2pool.tile([G, R], F32)
            nc.tensor.matmul(ps2[:], w_sb[0:HP, HP:HP + G],
                             tl[0:HP, hk0:hk0 + R],
                             start=True, stop=True)
            nc.scalar.copy(tl[0:G, hk0 + R:hk0 + 2 * R], ps2[:])
            # d = y - pred ; partials[g] = sum_r d^2
            d_sb = fpool.tile([G, R], F32)
            nc.vector.scalar_tensor_tensor(d_sb[:], ps2[:], -1.0, y_sb[:],
                                           mybir.AluOpType.mult,
                                           mybir.AluOpType.add)
            d2_sb = fpool.tile([G, R], F32)
            nc.vector.tensor_tensor(d2_sb[:], d_sb[:], d_sb[:],
                                    mybir.AluOpType.mult)
            nc.vector.tensor_reduce(
                tl[0:G, hk0 + 2 * R:hk0 + 2 * R + 1], d2_sb[:],
                mybir.AxisListType.X, mybir.AluOpType.add)

            nc.sync.dma_start(out_d[:], tl[0:HP, hk0:hk0 + 2 * R + 1])

    nc.compile()
    return nc


def _pack_weights(w_ih, w_hh, w_ho):
    """Fold params into the [NP, HP+G] stationary operand (see module doc)."""
    w = np.zeros((NP, HP + G), np.float32)
    dg = np.arange(G)
    for i in range(HIDDEN):
        for j in range(HIDDEN):
            w[i * G + dg, j * G + dg] = w_hh[i, j]
        w[HP + dg, i * G + dg] = w_ih[0, i]
        w[i * G + dg, HP + dg] = w_ho[i, 0]
    return w


def _host_scan(x_cols, w_ih, w_hh, h0=None):
    """fp32 numpy scan over the given columns (n_rows, n_cols)."""
    n = x_cols.shape[0]
    h = np.zeros((n, HIDDEN), np.float32) if h0 is None else h0
    for t in range(x_cols.shape[1]):
        h = np.tanh(x_cols[:, t:t + 1] * w_ih[0] + h @ w_hh).astype(np.float32)
    return h


def _choose_k(x_t, w_ih, w_hh, w_ho):
    """Smallest k with truncation error below fp32 noise, provably backed."""
    sigma = float(np.linalg.svd(w_hh.astype(np.float64), compute_uv=False)[0])
    if sigma >= 0.995:
        return T  # not provably contracting: run the full scan
    # provable bound: |pred err| <= ||w_ho|| * sigma^k * sqrt(3)
    who = float(np.linalg.norm(w_ho))
    k_bound = int(np.ceil(np.log(1e-8 / max(who * np.sqrt(3.0), 1e-30))
                          / np.log(sigma)))
    k_bound = max(16, min(T, k_bound))
    # empirical refinement on a row sample: accept k when starting 64 steps
    # earlier changes the sampled preds by < 2e-6 absolute (comparable to the
    # accumulated device tanh-table error, orders of magnitude below any
    # meaningful tolerance)
    rows = np.arange(0, B, max(1, B // 4096))
    for k in (24, 28, 32, 48, 64, 96, 128, 192, 256, 384, 512):
        if k >= k_bound:
            break
        ka = min(T, k + 64)
        pa = _host_scan(x_t[rows, T - k:], w_ih, w_hh) @ w_ho
        pb = _host_scan(x_t[rows, T - ka:], w_ih, w_hh) @ w_ho
        if float(np.abs(pa - pb).max()) < 2e-6:
            return k
    return k_bound


def kernel(x_t, y_t, w_ih, w_hh, w_ho):
    x_t = np.ascontiguousarray(x_t, np.float32)
    y_t = np.asarray(y_t, np.float32)
    w_ih = np.asarray(w_ih, np.float32)
    w_hh = np.asarray(w_hh, np.float32)
    w_ho = np.asarray(w_ho, np.float32)

    k_total = _choose_k(x_t, w_ih, w_hh, w_ho)
    n_seg = max(1, -(-k_total // KMAX))
    k_seg = -(-k_total // n_seg)
    k_seg = -(-k_seg // 4) * 4  # pad to multiple of 4
    span = n_seg * k_seg          # total steps incl. zero-pad (h stays 0 there)

    chained = n_seg > 1
    if ("k", k_seg, chained) not in _cache:
        _cache[("k", k_seg, chained)] = _build(k_seg, chained)
    nc = _cache[("k", k_seg, chained)]

    w_pack = _pack_weights(w_ih, w_hh, w_ho)
    trace = os.environ.get("BASS_RNN_TRACE", "0") == "1"
    if trace:
        trace = _install_ntff_shim()

    # columns T-span .. T, zero-padded on the left if span > T
    if span > T:
        x_cols = np.concatenate(
            [np.zeros((B, span - T), np.float32), x_t], axis=1)
    else:
        x_cols = x_t[:, T - span:]

    h = np.zeros((HP, R), np.float32)
    h_per_core = [h] * NCORES
    global last_exec_time_ns, last_profile
    res = None
    for s in range(n_seg):
        in_maps = []
        for c in range(NCORES):
            rows = slice(c * ROWS, (c + 1) * ROWS)
            xc = x_cols[rows, s * k_seg:(s + 1) * k_seg]
            # [2048, k] -> [g, t, r] layout
            xc = np.ascontiguousarray(
                xc.reshape(G, R, k_seg).transpose(0, 2, 1))
            im = {
                "x": xc,
                "y": np.ascontiguousarray(y_t[rows].reshape(G, R)),
                "w": w_pack,
            }
            if chained:
                im["h0"] = h_per_core[c]
            in_maps.append(im)
        res = run_bass_kernel_spmd(nc, in_maps, list(range(NCORES)),
                                   trace=trace and s == n_seg - 1)
        h_per_core = [np.ascontiguousarray(
            np.asarray(res.results[c]["out"])[:, 0:R]) for c in range(NCORES)]
    last_exec_time_ns = res.exec_time_ns
    last_profile = res.profile_json

    pred = np.empty(B, np.float32)
    tot = 0.0
    for c in range(NCORES):
        o = np.asarray(res.results[c]["out"])
        pred[c * ROWS:(c + 1) * ROWS] = o[0:G, R:2 * R].reshape(-1)
        tot += float(o[0:G, 2 * R].sum(dtype=np.float64))
    loss = np.float32(tot / B)
    return loss, pred
